# revision 2
# baseline (speedup 1.0000x reference)
"""BiLSTM-CRF loss kernel for trn2, one core = 32 sequences (data parallel).

- embedding lookup: plain (non-transpose) dma_gather -> token-major xtok,
  then per-128-token hardware DMA transposes (xbar) into x (E, ntok); gathers
  stream middle-out so both LSTM directions start immediately
- BiLSTM: all-sigmoid gates (tanh folded into 2x-scaled g-gate), gates psum:
  per-window xproj matmuls + bf16 rank-1 bias matmuls + per-step Whh matmuls;
  f/b chains emitted with a 2-step skew
- emissions (exp space, chunked middle-out) feed two concurrently-interleaved
  CRF chains: alpha (exp-space forward) and an augmented 50-row G chain whose
  extra rows carry the constant-1 and end_trans*laststep injection inside the
  single 50x50 matmul per round
- numerator transition part (start/trans/end/bout histogram) and
  masksum*log(T) are computed on host in numpy; the device returns the
  emission-gather numerator part A = sum_masked (Wout h)[tag] and the
  denominator log-sum
"""
import numpy as np
import ml_dtypes

import concourse.bacc as bacc
import concourse.mybir as mybir
from concourse.tile import TileContext

BF16 = ml_dtypes.bfloat16
F32 = np.float32
AF = mybir.ActivationFunctionType
ALU = mybir.AluOpType
DT = mybir.dt

T = 48


# --------------------------------------------------------------------------
# host-side preparation
# --------------------------------------------------------------------------

def prep_params(inp):
    """Build replicated parameter arrays (numpy) from raw inputs."""
    p = {}
    p["emb"] = np.ascontiguousarray(inp["emb"]).astype(BF16)

    def mk(Wih, Whh, bih, bhh):
        def reorder(W):
            i, f, g, o = np.split(np.asarray(W, F32), 4, 0)
            return np.concatenate([i, f, o, 2.0 * g], 0)
        WihT = np.ascontiguousarray(reorder(Wih).T).astype(BF16)   # (128, 512)
        WhhT = np.ascontiguousarray(reorder(Whh).T).astype(BF16)   # (128, 512)
        b = np.asarray(bih, F32) + np.asarray(bhh, F32)
        bi, bf_, bg, bo = np.split(b, 4)
        bias = np.concatenate([bi, bf_, bo, 2.0 * bg]).reshape(4, 128).astype(BF16)
        return WihT, WhhT, bias

    p["wiht_f"], p["whht_f"], bias_f = mk(inp["Wih_f"], inp["Whh_f"], inp["bih_f"], inp["bhh_f"])
    p["wiht_b"], p["whht_b"], bias_b = mk(inp["Wih_b"], inp["Whh_b"], inp["bih_b"], inp["bhh_b"])
    for gi in range(4):
        p[f"bias_f{gi}"] = np.ascontiguousarray(bias_f[gi:gi + 1])
        p[f"bias_b{gi}"] = np.ascontiguousarray(bias_b[gi:gi + 1])
    Wout = np.asarray(inp["Wout"], F32)     # (48, 256)
    H = Wout.shape[1] // 2
    p["wot_f"] = np.ascontiguousarray(Wout[:, :H].T).astype(BF16)   # (128, 48)
    p["wot_b"] = np.ascontiguousarray(Wout[:, H:].T).astype(BF16)
    c0 = np.log(T)
    p["exbias"] = (np.asarray(inp["bout"], F32) - c0).reshape(T, 1).astype(F32)
    trans = np.asarray(inp["trans"], F32)
    p["et"] = np.exp(trans).astype(BF16)                     # (48,48) lhsT alpha
    # augmented G-chain lhsT (50, 50):
    #   out[m<48] = sum_j ett[j,m] emg[j] + eend[m] * emg[49]
    #   out[48] = emg[48] (constant-1 carrier), out[49] = emg[48]
    ett50 = np.zeros((50, 50), F32)
    ett50[0:48, 0:48] = np.exp(trans).T      # lhsT[k, m] = e^{trans[m, k]}
    ett50[49, 0:48] = np.exp(np.asarray(inp["end_trans"], F32))
    ett50[48, 48] = 1.0
    ett50[48, 49] = 1.0
    p["ett50"] = ett50.astype(BF16)
    p["estart"] = np.exp(np.asarray(inp["start_trans"], F32)).reshape(T, 1).astype(F32)
    # rank-2 bias broadcast selector: bsel[k, n] = 1 iff n // REG == k
    REG = 256
    bsel = np.zeros((2, 2 * REG), np.float32)
    for k in range(2):
        bsel[k, k * REG:(k + 1) * REG] = 1.0
    p["bsel"] = bsel.astype(BF16)
    # G-chain init lhsT (2, 50): out = eend*ls_{L-1} on rows 0-47, 1 on rows 48-49
    ginit = np.zeros((2, 50), F32)
    ginit[0, 0:48] = np.exp(np.asarray(inp["end_trans"], F32))
    ginit[1, 48:50] = 1.0
    p["ginit"] = ginit.astype(BF16)
    p["iota48c"] = np.arange(T, dtype=F32).reshape(T, 1)
    return p


def prep_shard(words, tags, mask, inp):
    """Per-core input arrays (numpy) + host-side scalar partials."""
    b, L = words.shape
    ntok = b * L
    w_tm = np.ascontiguousarray(words.T).reshape(-1)
    m_tm = np.ascontiguousarray(mask.T).reshape(-1).astype(F32)
    tags_tm = np.ascontiguousarray(tags.T).reshape(-1)

    d = {}
    gi = w_tm.astype(np.int16).reshape(ntok // 16, 16).T          # (16, ntok/16)
    d["gidx"] = np.ascontiguousarray(np.tile(gi, (8, 1))).astype(np.int16)
    tm_masked = np.where(m_tm > 0, tags_tm, 99).astype(F32)
    d["tmask"] = tm_masked.astype(BF16).reshape(1, ntok)
    m_pad = np.pad(m_tm, (0, b))
    ls = (m_tm - m_pad[b:]).astype(BF16)
    # EM rows 48 (const 1) and 49 (ls shifted right by one step)
    emtop = np.ones((2, ntok), BF16)
    emtop[1, :b] = 0
    emtop[1, b:] = ls[:-b]
    d["emtop"] = emtop
    # G init rhs (2, BLOC): row0 = ls at t = L-1, row1 = ones
    ginitrhs = np.ones((2, b), BF16)
    ginitrhs[0] = ls[-b:]
    d["ginitrhs"] = ginitrhs

    # ---- host-side numerator transition part + masksum
    mF = mask.astype(np.float64)          # (b, L)
    tg = tags.astype(np.int64)
    trans = np.asarray(inp["trans"], np.float64)
    start_trans = np.asarray(inp["start_trans"], np.float64)
    end_trans = np.asarray(inp["end_trans"], np.float64)
    bout = np.asarray(inp["bout"], np.float64)
    num = start_trans[tg[:, 0]].sum()
    num += (trans[tg[:, :-1], tg[:, 1:]] * mF[:, 1:]).sum()
    num += (bout[tg] * mF).sum()
    seq_ends = mF.sum(1).astype(np.int64) - 1
    num += end_trans[tg[np.arange(b), seq_ends]].sum()
    scal = {"hostnum": float(num), "masksum": float(mF.sum())}
    return d, scal


# --------------------------------------------------------------------------
# device kernel builder
# --------------------------------------------------------------------------

def build(L=512, BLOC=32, W=8, V=32000):
    ntok = L * BLOC
    NW = L // W
    half = L // 2
    NCH = ntok // 512          # emission chunks

    nc = bacc.Bacc()
    dp = nc.declare_dram_parameter
    g_gidx = dp("gidx", [128, ntok // 16], DT.int16, isOutput=False)
    g_tmask = dp("tmask", [1, ntok], DT.bfloat16, isOutput=False)
    g_emtop = dp("emtop", [2, ntok], DT.bfloat16, isOutput=False)
    g_ginitrhs = dp("ginitrhs", [2, BLOC], DT.bfloat16, isOutput=False)
    g_emb = dp("emb", [V, 128], DT.bfloat16, isOutput=False)
    g_w = {}
    for nm in ("wiht_f", "whht_f", "wiht_b", "whht_b"):
        g_w[nm] = dp(nm, [128, 512], DT.bfloat16, isOutput=False)
    g_bias = {(d, gi): dp(f"bias_{d}{gi}", [1, 128], DT.bfloat16, isOutput=False)
              for d in "fb" for gi in range(4)}
    g_bsel = dp("bsel", [2, 2 * 32 * W], DT.bfloat16, isOutput=False)
    g_wot = {d: dp(f"wot_{d}", [128, T], DT.bfloat16, isOutput=False) for d in "fb"}
    g_exbias = dp("exbias", [T, 1], DT.float32, isOutput=False)
    g_et = dp("et", [T, T], DT.bfloat16, isOutput=False)
    g_ett50 = dp("ett50", [50, 50], DT.bfloat16, isOutput=False)
    g_estart = dp("estart", [T, 1], DT.float32, isOutput=False)
    g_ginit = dp("ginit", [2, 50], DT.bfloat16, isOutput=False)
    g_iota48c = dp("iota48c", [T, 1], DT.float32, isOutput=False)
    g_out = dp("out", [1, 8], DT.float32, isOutput=True)

    with TileContext(nc) as tc:
        with tc.tile_pool(name="persist", bufs=1) as pp:
            # ---- persistent SBUF tiles
            Hf = pp.tile([128, ntok], DT.bfloat16, tag="Hf", name="Hf")
            Hb = pp.tile([128, ntok], DT.bfloat16, tag="Hb", name="Hb")
            wiht = {}
            whht = {}
            bias = {}
            wot = {}
            for d in "fb":
                wiht[d] = pp.tile([128, 512], DT.bfloat16, tag=f"wiht{d}", name=f"wiht{d}")
                whht[d] = pp.tile([128, 512], DT.bfloat16, tag=f"whht{d}", name=f"whht{d}")
                for gi in range(4):
                    bias[d, gi] = pp.tile([1, 128], DT.bfloat16, tag=f"bias{d}{gi}", name=f"bias{d}{gi}")
                wot[d] = pp.tile([128, T], DT.bfloat16, tag=f"wot{d}", name=f"wot{d}")
            exbias = pp.tile([T, 1], DT.float32, tag="exbias", name="exbias")
            et_sb = pp.tile([T, T], DT.bfloat16, tag="et", name="et")
            ett50 = pp.tile([50, 50], DT.bfloat16, tag="ett50", name="ett50")
            estart = pp.tile([T, 1], DT.float32, tag="estart", name="estart")
            ginit = pp.tile([2, 50], DT.bfloat16, tag="ginit", name="ginit")
            ginitrhs = pp.tile([2, BLOC], DT.bfloat16, tag="ginitrhs", name="ginitrhs")
            iota48c = pp.tile([T, 1], DT.float32, tag="iota48c", name="iota48c")
            tmask_sb = pp.tile([1, ntok], DT.bfloat16, tag="tmask", name="tmask")
            # small constants
            ones48row = pp.tile([1, T], DT.float32, tag="ones48row", name="ones48row")
            ones48rowb = pp.tile([1, T], DT.bfloat16, tag="ones48rowb", name="ones48rowb")
            bsel = pp.tile([2, 2 * 32 * W], DT.bfloat16, tag="bsel", name="bsel")
            ones48col = pp.tile([T, 1], DT.float32, tag="ones48col", name="ones48col")
            # LSTM state (bf16: enables DVE 2x/4x packed modes)
            cst = {d: pp.tile([128, BLOC], DT.bfloat16, tag=f"c{d}", name=f"c{d}") for d in "fb"}
            tmp1 = {d: pp.tile([128, BLOC], DT.bfloat16, tag=f"tmp1{d}", name=f"tmp1{d}") for d in "fb"}
            tmp2 = {d: pp.tile([128, BLOC], DT.bfloat16, tag=f"tmp2{d}", name=f"tmp2{d}") for d in "fb"}
            tct = {d: pp.tile([128, BLOC], DT.bfloat16, tag=f"tct{d}", name=f"tct{d}") for d in "fb"}
            jacc = {d: pp.tile([128, 1], DT.float32, tag=f"jacc{d}", name=f"jacc{d}") for d in "fb"}
            # numerator accumulators
            accA = pp.tile([T, NCH], DT.float32, tag="accA", name="accA")
            accA_red = pp.tile([T, 1], DT.float32, tag="accAred", name="accAred")
            junkA = pp.tile([T, 512], DT.bfloat16, tag="junkA", name="junkA")
            # CRF tiles
            ea = [pp.tile([T, BLOC], DT.bfloat16, tag=f"ea{i}", name=f"ea{i}") for i in range(2)]
            emg = pp.tile([50, BLOC], DT.bfloat16, tag="emg", name="emg")
            dott = pp.tile([T, BLOC], DT.float32, tag="dott", name="dott")
            logrow = pp.tile([1, BLOC], DT.float32, tag="logrow", name="logrow")
            dsum = pp.tile([1, 1], DT.float32, tag="dsum", name="dsum")
            out_sb = pp.tile([1, 8], DT.float32, tag="outsb", name="outsb")
            # EMhat: rows 0-47 emission exp, row 48 = 1, row 49 = lsshift
            EM = pp.tile([50, ntok], DT.bfloat16, tag="EM", name="EM")

            # ---- input DMAs
            S = nc.sync
            for d in "fb":
                S.dma_start(out=wiht[d][:], in_=g_w[f"wiht_{d}"][:])
                S.dma_start(out=whht[d][:], in_=g_w[f"whht_{d}"][:])
                for gi in range(4):
                    S.dma_start(out=bias[d, gi][:], in_=g_bias[d, gi][:])
                S.dma_start(out=wot[d][:], in_=g_wot[d][:])
            S.dma_start(out=exbias[:], in_=g_exbias[:])
            S.dma_start(out=et_sb[:], in_=g_et[:])
            S.dma_start(out=ett50[:], in_=g_ett50[:])
            S.dma_start(out=estart[:], in_=g_estart[:])
            S.dma_start(out=bsel[:], in_=g_bsel[:])
            S.dma_start(out=ginit[:], in_=g_ginit[:])
            S.dma_start(out=ginitrhs[:], in_=g_ginitrhs[:])
            S.dma_start(out=iota48c[:], in_=g_iota48c[:])
            S.dma_start(out=tmask_sb[:], in_=g_tmask[:])
            S.dma_start(out=EM[48:50, :], in_=g_emtop[:])

            # constants
            Vv = nc.vector
            Sc = nc.scalar
            Vv.memset(ones48row[:], 1.0)
            Vv.memset(ones48rowb[:], 1.0)
            Vv.memset(ones48col[:], 1.0)
            Vv.memset(accA[:], 0.0)
            Vv.memset(out_sb[:], 0.0)
            for d in "fb":
                Vv.memset(cst[d][:], 0.0)

            # ---------------- LSTM ----------------
            REG = 32 * W      # region width per gate
            Hdir = {"f": Hf, "b": Hb}
            with tc.tile_pool(name="lstm_ps", bufs=2, space="PSUM") as lpsp, \
                 tc.tile_pool(name="lstm_sb", bufs=3) as lsb, \
                 tc.tile_pool(name="xpool", bufs=1) as xp:
                x = xp.tile([128, ntok], DT.bfloat16, tag="x", name="x")
                xtok = xp.tile([128, ntok // 128, 128], DT.bfloat16, tag="xtok",
                               name="xtok")
                gidx = xp.tile([128, ntok // 16], DT.int16, tag="gidx", name="gidx")
                S.dma_start(out=gidx[:], in_=g_gidx[:])

                def gather(tok0, tok1):
                    # plain (non-transpose) gather: 1 descriptor per token,
                    # then per-128-token hardware DMA transposes into x layout
                    n = tok1 - tok0
                    nc.gpsimd.dma_gather(
                        out_ap=xtok[:, tok0 // 128:tok1 // 128, :],
                        in_ap=g_emb[:],
                        idxs_ap=gidx[:, tok0 // 16:tok1 // 16],
                        num_idxs=n,
                        num_idxs_reg=n,
                        elem_size=128,
                        transpose=False,
                        single_packet=False,
                    )
                    for g in range(tok0 // 128, tok1 // 128):
                        S.dma_start_transpose(
                            out=x[:, g * 128:(g + 1) * 128],
                            in_=xtok[:, g, :])

                # small head gathers so window 0 starts fast, then 1024-token
                # chunks middle-out (f consumes from the front, b from the back)
                gather(0, 256)
                gather(ntok - 256, ntok)
                gather(256, 1024)
                gather(ntok - 1024, ntok - 256)
                _ng = ntok // 1024
                for _i in range(1, (_ng + 1) // 2):
                    gather(_i * 1024, (_i + 1) * 1024)
                    gather(ntok - (_i + 1) * 1024, ntok - _i * 1024)
                pfprev = None
                for w in range(NW):
                    pf = {}
                    for d in "fb":
                        pf[d] = lpsp.tile([128, 4 * REG], DT.float32, tag=f"pf{d}", name=f"pf{d}")
                        if d == "f":
                            x0 = w * W * BLOC
                        else:
                            x0 = (L - (w + 1) * W) * BLOC
                        for gi in range(4):
                            nc.tensor.matmul(
                                pf[d][:, gi * REG:(gi + 1) * REG],
                                wiht[d][:, gi * 128:(gi + 1) * 128],
                                x[:, x0:x0 + W * BLOC],
                                start=(gi % 2 == 0),
                                stop=False, skip_group_check=True)
                        for gi in range(4):
                            nc.tensor.matmul(
                                pf[d][:, gi * REG:(gi + 1) * REG],
                                bias[d, gi][:],
                                bsel[0:1, 0:REG],
                                start=False, stop=False, skip_group_check=True)
                    LAG = 2
                    for sl_ in range(W):
                        for d in "fb":
                            if d == "f":
                                s = sl_
                                t = w * W + s
                                slot = s
                                tprev_col = (t - 1) * BLOC
                                first = (t == 0)
                                pfd = pf[d]
                            else:
                                if sl_ < LAG:
                                    # tail steps of the previous window's b chain
                                    if w == 0:
                                        continue
                                    s = W - LAG + sl_
                                    t = L - 1 - ((w - 1) * W + s)
                                    pfd = pfprev
                                else:
                                    s = sl_ - LAG
                                    t = L - 1 - (w * W + s)
                                    pfd = pf[d]
                                slot = W - 1 - s
                                tprev_col = (t + 1) * BLOC
                                first = (t == L - 1)
                            Hd = Hdir[d]
                            if not first:
                                for gi in range(4):
                                    nc.tensor.matmul(
                                        pfd[:, gi * REG + slot * 32: gi * REG + (slot + 1) * 32],
                                        whht[d][:, gi * 128:(gi + 1) * 128],
                                        Hd[:, tprev_col:tprev_col + BLOC],
                                        start=False, stop=True, skip_group_check=True)
                            # sigma over the 4 gate slices
                            Sg = lsb.tile([128, 128], DT.bfloat16, tag=f"S{d}", name=f"S{d}")
                            pf3 = pfd[:].rearrange("p (g n) -> p g n", g=4)
                            Sc.activation(
                                Sg[:].rearrange("p (g n) -> p g n", g=4),
                                pf3[:, :, slot * 32:(slot + 1) * 32],
                                AF.Sigmoid)
                            # c update
                            if first:
                                Vv.affine_mul_reduce(
                                    out=tmp2[d][:], accum_out=jacc[d][:],
                                    in0=Sg[:, 96:128], in1=Sg[:, 0:32],
                                    scale=2.0, bias=-1.0)
                                Vv.tensor_copy(cst[d][:], tmp2[d][:])
                            else:
                                Vv.tensor_tensor(
                                    out=tmp1[d][:], in0=Sg[:, 32:64],
                                    in1=cst[d][:], op=ALU.mult)
                                Vv.affine_mul_reduce(
                                    out=tmp2[d][:], accum_out=jacc[d][:],
                                    in0=Sg[:, 96:128], in1=Sg[:, 0:32],
                                    scale=2.0, bias=-1.0)
                                Vv.tensor_tensor(out=cst[d][:], in0=tmp1[d][:],
                                                 in1=tmp2[d][:], op=ALU.add)
                            Sc.activation(tct[d][:], cst[d][:], AF.Tanh)
                            Vv.tensor_tensor(
                                out=Hd[:, t * BLOC:(t + 1) * BLOC],
                                in0=Sg[:, 64:96], in1=tct[d][:],
                                op=ALU.mult)

                    pfprev = pf["b"]
                # flush the final LAG steps of the b chain (window NW-1)
                for s in range(W - 2, W):
                    d = "b"
                    t = L - 1 - ((NW - 1) * W + s)
                    slot = W - 1 - s
                    tprev_col = (t + 1) * BLOC
                    first = (t == L - 1)
                    Hd = Hdir[d]
                    pfd = pfprev
                    if not first:
                        for gi in range(4):
                            nc.tensor.matmul(
                                pfd[:, gi * REG + slot * 32: gi * REG + (slot + 1) * 32],
                                whht[d][:, gi * 128:(gi + 1) * 128],
                                Hd[:, tprev_col:tprev_col + BLOC],
                                start=False, stop=True, skip_group_check=True)
                    Sg = lsb.tile([128, 128], DT.bfloat16, tag=f"S{d}", name=f"S{d}")
                    pf3 = pfd[:].rearrange("p (g n) -> p g n", g=4)
                    Sc.activation(
                        Sg[:].rearrange("p (g n) -> p g n", g=4),
                        pf3[:, :, slot * 32:(slot + 1) * 32],
                        AF.Sigmoid)
                    Vv.tensor_tensor(out=tmp1[d][:], in0=Sg[:, 32:64],
                                     in1=cst[d][:], op=ALU.mult)
                    Vv.affine_mul_reduce(
                        out=tmp2[d][:], accum_out=jacc[d][:],
                        in0=Sg[:, 96:128], in1=Sg[:, 0:32],
                        scale=2.0, bias=-1.0)
                    Vv.tensor_tensor(out=cst[d][:], in0=tmp1[d][:],
                                     in1=tmp2[d][:], op=ALU.add)
                    Sc.activation(tct[d][:], cst[d][:], AF.Tanh)
                    Vv.tensor_tensor(out=Hd[:, t * BLOC:(t + 1) * BLOC],
                                     in0=Sg[:, 64:96], in1=tct[d][:],
                                     op=ALU.mult)

            # ---------------- emissions + CRF, interleaved ----------------
            with tc.tile_pool(name="em_ps", bufs=2, space="PSUM") as epsp, \
                 tc.tile_pool(name="crf_ps", bufs=1, space="PSUM") as kpsp, \
                 tc.tile_pool(name="fin_ps", bufs=1, space="PSUM") as fpsp, \
                 tc.tile_pool(name="em_sb", bufs=3) as esb:

                def emit_chunk(k):
                    cs = k * 512
                    emps = epsp.tile([T, 512], DT.float32, tag="emps", name="emps")
                    nc.tensor.matmul(emps[:], wot["f"][:], Hf[:, cs:cs + 512],
                                     start=True, stop=False, skip_group_check=True)
                    nc.tensor.matmul(emps[:], wot["b"][:], Hb[:, cs:cs + 512],
                                     start=False, stop=True, skip_group_check=True)
                    Sc.activation(EM[0:T, cs:cs + 512], emps[:], AF.Exp,
                                  bias=exbias[:])
                    # A-part: one-hot(tags) dot raw emission psum
                    tgps = epsp.tile([T, 512], DT.float32, tag="tgps", name="tgps")
                    nc.tensor.matmul(tgps[:], ones48rowb[:],
                                     tmask_sb[0:1, cs:cs + 512], start=True, stop=True,
                                     skip_group_check=True)
                    ohm = esb.tile([T, 512], DT.bfloat16, tag="ohm", name="ohm")
                    Vv.tensor_scalar(ohm[:], tgps[:], iota48c[:], None, ALU.is_equal)
                    Vv.affine_mul_reduce(
                        out=junkA[:], accum_out=accA[:, k:k + 1],
                        in0=emps[:], in1=ohm[:],
                        scale=1.0, bias=0.0)

                # first two chunks gate the chains
                emit_chunk(0)
                emit_chunk(NCH - 1)
                # alpha init (t=0)
                Vv.tensor_scalar(ea[0][:], EM[0:T, 0:BLOC], estart[:], None, ALU.mult)
                cur = 0
                # G init at t = L-1: rows 0-47 = eend * ls_{L-1}; rows 48,49 = 1
                gps_prev = kpsp.tile([50, BLOC], DT.float32, tag="pg", name="pg")
                nc.tensor.matmul(gps_prev[:], ginit[:], ginitrhs[:],
                                 start=True, stop=True, skip_group_check=True)

                # interleaved rounds; chunk pair (j, NCH-1-j) emitted ahead
                at = 1            # next alpha round t (reads EM chunk t//16)
                gt = L - 2        # next G round t (reads EM chunk (t+1)//16)
                CHW = 512 // BLOC   # tokens per chunk = 16 steps
                for j in range(1, NCH // 2 + 1):
                    if j < NCH // 2:
                        emit_chunk(j)
                        emit_chunk(NCH - 1 - j)
                    amax = min((j + 1) * CHW, half) if j < NCH // 2 else half
                    gmin = (L - (j + 1) * CHW) if j < NCH // 2 else (half - 1)
                    while at < amax or gt >= gmin:
                        if at < amax:
                            pa = kpsp.tile([T, BLOC], DT.float32, tag="pa", name="pa")
                            nc.tensor.matmul(pa[:], et_sb[:], ea[cur][:], start=True,
                                             stop=True, skip_group_check=True)
                            cur ^= 1
                            Vv.tensor_tensor(out=ea[cur][:], in0=pa[:],
                                             in1=EM[0:T, at * BLOC:(at + 1) * BLOC],
                                             op=ALU.mult)
                            at += 1
                        if gt >= gmin:
                            Vv.tensor_tensor(
                                out=emg[:], in0=gps_prev[:],
                                in1=EM[:, (gt + 1) * BLOC:(gt + 2) * BLOC],
                                op=ALU.mult)
                            gps = kpsp.tile([50, BLOC], DT.float32, tag="pg", name="pg")
                            nc.tensor.matmul(gps[:], ett50[:], emg[:], start=True,
                                             stop=True, skip_group_check=True)
                            gps_prev = gps
                            gt -= 1

                # combine: alpha_{255} . G_{255}
                Vv.tensor_tensor(out=dott[:], in0=gps_prev[0:T, :], in1=ea[cur][:],
                                 op=ALU.mult)
                fint = fpsp.tile([1, 64], DT.float32, name="fint")
                nc.tensor.matmul(fint[:, 0:BLOC], ones48col[:], dott[:], start=True,
                                 stop=True, skip_group_check=True)
                Sc.activation(logrow[:], fint[:, 0:BLOC], AF.Ln)
                Vv.tensor_reduce(dsum[:], logrow[:], mybir.AxisListType.X, ALU.add)

                # numerator A total
                Vv.tensor_reduce(accA_red[:], accA[:], mybir.AxisListType.X, ALU.add)
                nc.tensor.matmul(fint[:, 34:35], accA_red[:], ones48col[:], start=True,
                                 stop=True, skip_group_check=True)
                Vv.tensor_copy(out_sb[:, 0:1], fint[:, 34:35])
                Vv.tensor_copy(out_sb[:, 1:2], dsum[:])
            S.dma_start(out=g_out[:], in_=out_sb[:])

    return nc


# --------------------------------------------------------------------------
# self-contained entry point: kernel(**inputs) -> scalar loss (numpy)
# --------------------------------------------------------------------------

_CACHED = {}


def _get_nc():
    if "nc" not in _CACHED:
        nc = build(L=512, BLOC=32, W=8, V=32000)
        if not nc.is_finalized():
            nc.finalize()
        _CACHED["nc"] = nc
    return _CACHED["nc"]


def combine(results, scals, B=256):
    """results: list of out arrays (1,8); scals: list of host partial dicts."""
    c0 = float(np.log(T))
    tot_num = sum(float(r[0, 0]) for r in results) + sum(s["hostnum"] for s in scals)
    tot_den = sum(float(r[0, 1]) + c0 * s["masksum"]
                  for r, s in zip(results, scals))
    return np.float32((tot_den - tot_num) / B)


def kernel(**inputs):
    from concourse.bass_utils import run_bass_kernel_spmd

    B = 256
    BLOC = B // 8
    p = prep_params(inputs)
    in_maps = []
    scals = []
    words = np.asarray(inputs["words"])
    tags = np.asarray(inputs["tags"])
    mask = np.asarray(inputs["mask"])
    for core in range(8):
        sl = slice(core * BLOC, (core + 1) * BLOC)
        d, scal = prep_shard(words[sl], tags[sl], mask[sl], inputs)
        d.update(p)
        in_maps.append(d)
        scals.append(scal)
    nc = _get_nc()
    res = run_bass_kernel_spmd(nc, in_maps, list(range(8)))
    return combine([res.results[i]["out"] for i in range(8)], scals, B)
